# revision 4
# baseline (speedup 1.0000x reference)
"""Two-layer KAN (B-spline + silu base) fused Trainium2 kernel, 8-core SPMD.

Math: cubic B-spline basis on uniform grid [-2.2, 2.2] (h=0.4) rewritten as
relu(u-m)^3 features (u = 2.5*x + 5.5, clamped at 12), with the 5-tap stencil
[1,-4,6,-4,1]/6 folded into the spline weights host-side. Each KAN layer
becomes one dense matmul over 13 feature blocks (12 relu^3 + silu base).

Sharding: layer 1 contraction(in_dim)-parallel across 8 cores; partial
y1 (128,256) ReduceScatter(add) -> each core owns 16 batch rows; layer 2
batch-parallel with full contraction; host concatenates the 8 (16,10) shards.
"""

import numpy as np
import concourse.bass as bass
import concourse.mybir as mybir
import concourse.tile as tile
from concourse.bass_utils import run_bass_kernel_spmd
from concourse.masks import make_identity
from concourse.vector_clock import ScopedClock

f32 = mybir.dt.float32
AF = mybir.ActivationFunctionType
OP = mybir.AluOpType

NC_CORES = 8
B, IN, H, OUT, NB = 128, 3072, 256, 10, 8
I_LOC = IN // NC_CORES          # 384
NF = 13                         # 12 relu^3 features + silu base block
K1 = I_LOC * NF                 # 4992
NK1 = K1 // 128                 # 39
B_LOC = B // NC_CORES           # 16
K2 = H * NF                     # 3328
NK2 = K2 // 128                 # 26
LAM = 1.0507009873554805
ALPHA = 1.6732632423543772
LA = LAM * ALPHA
STENCIL = (np.array([1.0, -4.0, 6.0, -4.0, 1.0]) / 6.0).astype(np.float32)

# walrus codegen rejects instructions carrying more than one sem wait at the
# TileContext exit drain; split it into a chain of single-wait drains.
_WAIT_LIMIT = 1


def _patched_drain_and_barrier(self, tick_clock, wait_clock):
    nc = self.nc
    drain_inst = nc.sync.drain()
    wait_clock.add_sem_waits(
        drain_inst.ins, ScopedClock({None: tick_clock.global_clock})
    )
    si = drain_inst.ins.sync_info
    waits = list(si.on_wait) if si and si.on_wait else []
    if len(waits) > _WAIT_LIMIT:
        si.on_wait = waits[:_WAIT_LIMIT]
        for ofs in range(_WAIT_LIMIT, len(waits), _WAIT_LIMIT):
            extra = nc.sync.drain()
            chunk = waits[ofs : ofs + _WAIT_LIMIT]
            if extra.ins.sync_info is None:
                extra.ins.sync_info = mybir.SyncInfo(on_update=[], on_wait=chunk)
            else:
                extra.ins.sync_info.on_wait = chunk
    nc.all_engine_barrier()
    assert self.sems is not None
    popped = nc._tile_sem_poison_stack.pop()
    assert popped is self._sem_poison
    nc.clear_and_free_semaphores(list(self.sems.allocated().values()))
    nc.all_engine_barrier()


tile.TileContext._drain_and_barrier = _patched_drain_and_barrier


def _legalize_waits(nc, limit=1):
    """Split any instruction carrying >limit sem waits: move the overflow onto
    no-op instructions inserted immediately before it on the same engine."""
    n = 0
    for bbw in nc.bb_map.values():
        bb = bbw.bb
        i = 0
        while i < len(bb.instructions):
            inst = bb.instructions[i]
            si = inst.sync_info
            waits = list(si.on_wait) if si and si.on_wait else []
            if len(waits) > limit:
                si.on_wait = waits[-limit:]
                overflow = waits[:-limit]
                for ofs in range(0, len(overflow), limit):
                    nop = mybir.InstNoOp(name=f"legwait-{n}", engine=inst.engine,
                                         debug=inst.debug, ins=[], outs=[])
                    nop.sync_info = mybir.SyncInfo(
                        on_update=[], on_wait=overflow[ofs : ofs + limit])
                    nc.register_instruction(nop, overwrite=True)
                    bb.instructions.insert(i, nop)
                    n += 1
                    i += 1
            i += 1
    return n


def _fold(coef, ssp):
    """(O,I,8) spline coefs + per-edge scale -> (O,I,12) relu^3 weights."""
    O, I, _ = coef.shape
    cs = (coef * ssp[:, :, None]).astype(np.float32)
    W = np.zeros((O, I, 12), np.float32)
    for g in range(NB):
        for d in range(5):
            W[:, :, g + d] += cs[:, :, g] * STENCIL[d]
    return W


def _build_program():
    nc = bass.Bass("TRN2", target_bir_lowering=False, debug=False,
                   num_devices=NC_CORES)
    xt_d = nc.dram_tensor("xt", [128, 3 * B], f32, kind="ExternalInput")
    w1_d = nc.dram_tensor("w1", [128, NK1 * H], f32, kind="ExternalInput")
    w2_d = nc.dram_tensor("w2", [128, NK2 * OUT], f32, kind="ExternalInput")
    yp_d = nc.dram_tensor("yp", [B_LOC, OUT], f32, kind="ExternalOutput")

    with tile.TileContext(nc) as tc:
        with (
            tc.tile_pool(name="constp", bufs=1) as constp,
            tc.tile_pool(name="xp", bufs=1) as xp,
            tc.tile_pool(name="fp", bufs=1) as fp,
            tc.tile_pool(name="wp", bufs=4) as wp,
            tc.tile_pool(name="sp", bufs=4) as sp,
            tc.tile_pool(name="l2p", bufs=1) as l2p,
            tc.tile_pool(name="ps1", bufs=1, space="PSUM") as ps1,
            tc.tile_pool(name="ps2", bufs=2, space="PSUM") as ps2,
            tc.tile_pool(name="dram", bufs=1, space="DRAM") as dram,
        ):
            # constants
            ident = constp.tile([128, 128], f32)
            make_identity(nc, ident)
            mbias = constp.tile([128, 12 * 2 * B_LOC], f32)  # (128, 384)
            for m in range(12):
                nc.vector.memset(mbias[:, 32 * m : 32 * (m + 1)], float(m))
            warm = constp.tile([1, 1], f32)

            # ---- layer 1: x^T load, u, features ----
            xt = xp.tile([128, 3 * 128], f32)
            nc.sync.dma_start(out=xt[:], in_=xt_d.ap())
            u = xp.tile([128, 3 * 128], f32)
            nc.vector.tensor_scalar(u[:], xt[:], 2.5, 5.5, OP.mult, OP.add)
            nc.vector.tensor_scalar(u[:], u[:], 12.0, None, OP.min)

            F = fp.tile([128, K1], f32)
            nc.scalar.activation(F[:, 12 * I_LOC :], xt[:], AF.Silu)
            for m in range(12):
                r = sp.tile([128, I_LOC], f32, tag="r")
                s = sp.tile([128, I_LOC], f32, tag="s")
                nc.vector.tensor_scalar(r[:], u[:], float(m), 0.0,
                                        OP.subtract, OP.max)
                nc.scalar.activation(s[:], r[:], AF.Square)
                nc.vector.tensor_tensor(F[:, I_LOC * m : I_LOC * (m + 1)],
                                        s[:], r[:], OP.mult)
            # pre-warm Exp table while matmuls run
            nc.scalar.activation(warm[:], xt[:1, :1], AF.Exp)

            # ---- layer 1 matmul: 39 accumulating chunks ----
            y1ps = ps1.tile([128, H], f32)
            for i in range(13):
                wt = wp.tile([128, 3 * H], f32, tag="w1")
                nc.sync.dma_start(
                    out=wt[:], in_=w1_d.ap()[:, 3 * H * i : 3 * H * (i + 1)])
                for s3 in range(3):
                    j = 3 * i + s3
                    nc.tensor.matmul(
                        y1ps[:],
                        F[:, 128 * j : 128 * (j + 1)],
                        wt[:, H * s3 : H * (s3 + 1)],
                        start=(j == 0),
                        stop=(j == NK1 - 1),
                    )
            y1sb = l2p.tile([128, H], f32)
            nc.vector.tensor_copy(y1sb[:], y1ps[:])

            # ---- ReduceScatter: each core gets its 16 batch rows ----
            y1p = dram.tile([B, H], f32)
            y1r = dram.tile([B_LOC, H], f32)
            nc.sync.dma_start(out=y1p[:], in_=y1sb[:])
            nc.gpsimd.collective_compute(
                "ReduceScatter",
                OP.add,
                replica_groups=[list(range(NC_CORES))],
                ins=[y1p.opt()],
                outs=[y1r.opt()],
            )
            y1in = l2p.tile([B_LOC, H], f32)
            nc.sync.dma_start(out=y1in[:], in_=y1r[:])

            # ---- transpose (16,256) -> packed (128, 32) o-major ----
            hpre = l2p.tile([128, 2 * B_LOC], f32)
            for t in range(2):
                pt = ps2.tile([128, B_LOC], f32, tag="tp")
                nc.tensor.transpose(pt[:], y1in[:, 128 * t : 128 * (t + 1)],
                                    ident[:B_LOC, :B_LOC])
                nc.vector.tensor_copy(hpre[:, B_LOC * t : B_LOC * (t + 1)],
                                      pt[:])

            # ---- selu: h = max(lam*y,0) + la*(exp(min(y,0)) - 1) ----
            W2C = 2 * B_LOC  # 32
            ymin = l2p.tile([128, W2C], f32)
            e1 = l2p.tile([128, W2C], f32)
            a1 = l2p.tile([128, W2C], f32)
            c1 = l2p.tile([128, W2C], f32)
            h2 = l2p.tile([128, W2C], f32)
            nc.vector.tensor_scalar(ymin[:], hpre[:], 0.0, None, OP.min)
            nc.scalar.activation(e1[:], ymin[:], AF.Exp)
            nc.vector.tensor_scalar(a1[:], hpre[:], LAM, 0.0, OP.mult, OP.max)
            nc.vector.tensor_scalar(c1[:], e1[:], LA, LA, OP.mult, OP.subtract)
            nc.vector.tensor_tensor(h2[:], a1[:], c1[:], OP.add)

            # ---- layer-2 features ----
            F2 = l2p.tile([128, K2 // 128 * B_LOC], f32)  # (128, 416)
            # silu(h) = h / (1 + exp(-h))
            e2 = l2p.tile([128, W2C], f32)
            d2 = l2p.tile([128, W2C], f32)
            nc.scalar.activation(e2[:], h2[:], AF.Exp, scale=-1.0)
            nc.vector.tensor_scalar(d2[:], e2[:], 1.0, None, OP.add)
            nc.vector.reciprocal(d2[:], d2[:])
            nc.vector.tensor_tensor(F2[:, 12 * W2C :], h2[:], d2[:], OP.mult)
            # u2 and batched relu^3 features over all 12 shifts
            u2 = l2p.tile([128, W2C], f32)
            nc.vector.tensor_scalar(u2[:], h2[:], 2.5, 5.5, OP.mult, OP.add)
            nc.vector.tensor_scalar(u2[:], u2[:], 12.0, None, OP.min)
            r2 = l2p.tile([128, 12 * W2C], f32)
            s2 = l2p.tile([128, 12 * W2C], f32)
            nc.vector.tensor_tensor(
                r2[:].rearrange("p (m c) -> p m c", m=12),
                u2[:].unsqueeze(1).broadcast_to((128, 12, W2C)),
                mbias[:].rearrange("p (m c) -> p m c", m=12),
                OP.subtract,
            )
            nc.vector.tensor_scalar(r2[:], r2[:], 0.0, None, OP.max)
            nc.vector.tensor_tensor(s2[:], r2[:], r2[:], OP.mult)
            nc.vector.tensor_tensor(F2[:, : 12 * W2C], s2[:], r2[:], OP.mult)

            # ---- layer-2 weights + matmul: 26 chunks -> (16, 10) ----
            w2sb = l2p.tile([128, NK2 * OUT], f32)  # (128, 260)
            nc.sync.dma_start(out=w2sb[:], in_=w2_d.ap())
            yps2 = ps2.tile([B_LOC, OUT], f32, tag="acc2")
            for j in range(NK2):
                nc.tensor.matmul(
                    yps2[:],
                    F2[:, B_LOC * j : B_LOC * (j + 1)],
                    w2sb[:, OUT * j : OUT * (j + 1)],
                    start=(j == 0),
                    stop=(j == NK2 - 1),
                )
            ysb = l2p.tile([B_LOC, OUT], f32)
            nc.vector.tensor_copy(ysb[:], yps2[:])
            nc.sync.dma_start(out=yp_d.ap(), in_=ysb[:])

    _legalize_waits(nc)
    return nc


_NC_CACHE = None


def _get_program():
    global _NC_CACHE
    if _NC_CACHE is None:
        _NC_CACHE = _build_program()
    return _NC_CACHE


def _prep_inputs(x, coef1, scale_base1, scale_sp1, coef2, scale_base2,
                 scale_sp2):
    W1q = _fold(coef1, scale_sp1)                      # (256, 3072, 12)
    W2q = _fold(coef2, scale_sp2)                      # (10, 256, 12)
    w2full = np.concatenate(
        [
            np.ascontiguousarray(W2q.transpose(2, 1, 0)).reshape(12 * H, OUT),
            np.ascontiguousarray(scale_base2.T).reshape(H, OUT),
        ],
        axis=0,
    )                                                   # (3328, 10)
    w2full = np.ascontiguousarray(
        w2full.reshape(NK2, 128, OUT).transpose(1, 0, 2)).reshape(128, NK2 * OUT)
    in_maps = []
    for c in range(NC_CORES):
        sl = slice(c * I_LOC, (c + 1) * I_LOC)
        w1c = np.concatenate(
            [
                np.ascontiguousarray(W1q[:, sl, :].transpose(2, 1, 0))
                .reshape(12 * I_LOC, H),
                np.ascontiguousarray(scale_base1[:, sl].T).reshape(I_LOC, H),
            ],
            axis=0,
        )                                               # (4992, 256)
        w1c = np.ascontiguousarray(
            w1c.reshape(NK1, 128, H).transpose(1, 0, 2)).reshape(128, NK1 * H)
        xtc = np.ascontiguousarray(
            x[:, sl].T.reshape(3, 128, B).transpose(1, 0, 2)).reshape(128, 3 * B)
        in_maps.append({"xt": xtc, "w1": w1c, "w2": w2full})
    return in_maps


def kernel(x, coef1, scale_base1, scale_sp1, coef2, scale_base2, scale_sp2,
           _trace=False, **_unused):
    x = np.asarray(x, np.float32)
    coef1 = np.asarray(coef1, np.float32)
    scale_base1 = np.asarray(scale_base1, np.float32)
    scale_sp1 = np.asarray(scale_sp1, np.float32)
    coef2 = np.asarray(coef2, np.float32)
    scale_base2 = np.asarray(scale_base2, np.float32)
    scale_sp2 = np.asarray(scale_sp2, np.float32)

    in_maps = _prep_inputs(x, coef1, scale_base1, scale_sp1, coef2,
                           scale_base2, scale_sp2)
    nc = _get_program()
    res = run_bass_kernel_spmd(nc, in_maps, list(range(NC_CORES)),
                               trace=_trace)
    out = np.concatenate([np.asarray(res.results[c]["yp"])
                          for c in range(NC_CORES)], axis=0)
    if _trace:
        return out, res
    return out


# revision 7
# speedup vs baseline: 1.0190x; 1.0190x over previous
"""Two-layer KAN (B-spline + silu base) fused Trainium2 kernel, 8-core SPMD.

Math: cubic B-spline basis on uniform grid [-2.2, 2.2] (h=0.4) rewritten as
relu(u-m)^3 features (u = 2.5*x + 5.5, clamped at 12), with the 5-tap stencil
[1,-4,6,-4,1]/6 folded into the spline weights host-side. Each KAN layer
becomes one dense matmul over 13 feature blocks (12 relu^3 + silu base).

Sharding: layer 1 contraction(in_dim)-parallel across 8 cores; partial
y1 (128,256) ReduceScatter(add) -> each core owns 16 batch rows; layer 2
batch-parallel with full contraction; host concatenates the 8 (16,10) shards.
"""

import numpy as np
import concourse.bass as bass
import concourse.mybir as mybir
import concourse.tile as tile
from concourse.bass_utils import run_bass_kernel_spmd
from concourse.masks import make_identity
from concourse.vector_clock import ScopedClock

f32 = mybir.dt.float32
f32r = mybir.dt.float32r
AF = mybir.ActivationFunctionType
OP = mybir.AluOpType

NC_CORES = 8
B, IN, H, OUT, NB = 128, 3072, 256, 10, 8
I_LOC = IN // NC_CORES          # 384
NF = 13                         # 12 relu^3 features + silu base block
K1 = I_LOC * NF                 # 4992
NK1 = K1 // 128                 # 39
B_LOC = B // NC_CORES           # 16
K2 = H * NF                     # 3328
NK2 = K2 // 128                 # 26
LAM = 1.0507009873554805
ALPHA = 1.6732632423543772
LA = LAM * ALPHA
STENCIL = (np.array([1.0, -4.0, 6.0, -4.0, 1.0]) / 6.0).astype(np.float32)

# walrus codegen rejects instructions carrying more than one sem wait at the
# TileContext exit drain; split it into a chain of single-wait drains.
_WAIT_LIMIT = 1


def _patched_drain_and_barrier(self, tick_clock, wait_clock):
    nc = self.nc
    drain_inst = nc.sync.drain()
    wait_clock.add_sem_waits(
        drain_inst.ins, ScopedClock({None: tick_clock.global_clock})
    )
    si = drain_inst.ins.sync_info
    waits = list(si.on_wait) if si and si.on_wait else []
    if len(waits) > _WAIT_LIMIT:
        si.on_wait = waits[:_WAIT_LIMIT]
        for ofs in range(_WAIT_LIMIT, len(waits), _WAIT_LIMIT):
            extra = nc.sync.drain()
            chunk = waits[ofs : ofs + _WAIT_LIMIT]
            if extra.ins.sync_info is None:
                extra.ins.sync_info = mybir.SyncInfo(on_update=[], on_wait=chunk)
            else:
                extra.ins.sync_info.on_wait = chunk
    nc.all_engine_barrier()
    assert self.sems is not None
    popped = nc._tile_sem_poison_stack.pop()
    assert popped is self._sem_poison
    nc.clear_and_free_semaphores(list(self.sems.allocated().values()))
    nc.all_engine_barrier()


tile.TileContext._drain_and_barrier = _patched_drain_and_barrier


def _legalize_waits(nc, limit=1):
    """Split any instruction carrying >limit sem waits: move the overflow onto
    no-op instructions inserted immediately before it on the same engine."""
    n = 0
    for bbw in nc.bb_map.values():
        bb = bbw.bb
        i = 0
        while i < len(bb.instructions):
            inst = bb.instructions[i]
            si = inst.sync_info
            waits = list(si.on_wait) if si and si.on_wait else []
            if len(waits) > limit:
                si.on_wait = waits[-limit:]
                overflow = waits[:-limit]
                for ofs in range(0, len(overflow), limit):
                    nop = mybir.InstNoOp(name=f"legwait-{n}", engine=inst.engine,
                                         debug=inst.debug, ins=[], outs=[])
                    nop.sync_info = mybir.SyncInfo(
                        on_update=[], on_wait=overflow[ofs : ofs + limit])
                    nc.register_instruction(nop, overwrite=True)
                    bb.instructions.insert(i, nop)
                    n += 1
                    i += 1
            i += 1
    return n


def _fold(coef, ssp):
    """(O,I,8) spline coefs + per-edge scale -> (O,I,12) relu^3 weights."""
    O, I, _ = coef.shape
    cs = (coef * ssp[:, :, None]).astype(np.float32)
    W = np.zeros((O, I, 12), np.float32)
    for g in range(NB):
        for d in range(5):
            W[:, :, g + d] += cs[:, :, g] * STENCIL[d]
    return W


def _build_program():
    nc = bass.Bass("TRN2", target_bir_lowering=False, debug=False,
                   num_devices=NC_CORES)
    xt_d = nc.dram_tensor("xt", [128, 3 * B], f32, kind="ExternalInput")
    w1_d = nc.dram_tensor("w1", [128, NK1 * H], f32, kind="ExternalInput")
    w2_d = nc.dram_tensor("w2", [128, NK2 * OUT], f32, kind="ExternalInput")
    yp_d = nc.dram_tensor("yp", [B_LOC, OUT], f32, kind="ExternalOutput")

    with tile.TileContext(nc) as tc:
        with (
            tc.tile_pool(name="constp", bufs=1) as constp,
            tc.tile_pool(name="xp", bufs=1) as xp,
            tc.tile_pool(name="fp", bufs=1) as fp,
            tc.tile_pool(name="wp", bufs=4) as wp,
            tc.tile_pool(name="sp", bufs=4) as sp,
            tc.tile_pool(name="l2p", bufs=1) as l2p,
            tc.tile_pool(name="ps1", bufs=1, space="PSUM") as ps1,
            tc.tile_pool(name="ps2", bufs=2, space="PSUM") as ps2,
            tc.tile_pool(name="dram", bufs=1, space="DRAM") as dram,
        ):
            # constants
            ident = constp.tile([128, 128], f32)
            make_identity(nc, ident)
            mbias = constp.tile([128, 12 * 2 * B_LOC], f32)  # (128, 384)
            for m in range(12):
                nc.vector.memset(mbias[:, 32 * m : 32 * (m + 1)], float(m))
            warm = constp.tile([1, 1], f32)

            # ---- layer 1: x^T load, u, features ----
            xt = xp.tile([128, 3 * 128], f32)
            nc.sync.dma_start(out=xt[:], in_=xt_d.ap())
            u = xp.tile([128, 3 * 128], f32)
            nc.vector.tensor_scalar(u[:], xt[:], 2.5, 5.5, OP.mult, OP.add)
            nc.vector.tensor_scalar(u[:], u[:], 12.0, None, OP.min)

            F = fp.tile([128, K1], f32r)
            nc.scalar.activation(F[:, 12 * I_LOC :], xt[:], AF.Silu)
            for m in range(12):
                r = sp.tile([128, I_LOC], f32, tag="r")
                s = sp.tile([128, I_LOC], f32, tag="s")
                nc.vector.tensor_scalar(r[:], u[:], float(m), 0.0,
                                        OP.subtract, OP.max)
                nc.scalar.activation(s[:], r[:], AF.Square)
                nc.vector.tensor_tensor(F[:, I_LOC * m : I_LOC * (m + 1)],
                                        s[:], r[:], OP.mult)
            # pre-warm Exp table while matmuls run
            nc.scalar.activation(warm[:], xt[:1, :1], AF.Exp)

            # ---- layer 1 matmul: 39 accumulating chunks ----
            y1ps = ps1.tile([128, H], f32)
            for i in range(13):
                wt = wp.tile([128, 3 * H], f32r, tag="w1")
                nc.sync.dma_start(
                    out=wt[:],
                    in_=w1_d.ap()[:, 3 * H * i : 3 * H * (i + 1)].bitcast(f32r))
                for s3 in range(3):
                    j = 3 * i + s3
                    nc.tensor.matmul(
                        y1ps[:],
                        F[:, 128 * j : 128 * (j + 1)],
                        wt[:, H * s3 : H * (s3 + 1)],
                        start=(j == 0),
                        stop=(j == NK1 - 1),
                    )
            y1sb = l2p.tile([128, H], f32)
            nc.vector.tensor_copy(y1sb[:], y1ps[:])

            # ---- ReduceScatter: each core gets its 16 batch rows ----
            y1p = dram.tile([B, H], f32)
            y1r = dram.tile([B_LOC, H], f32)
            nc.sync.dma_start(out=y1p[:], in_=y1sb[:])
            nc.gpsimd.collective_compute(
                "ReduceScatter",
                OP.add,
                replica_groups=[list(range(NC_CORES))],
                ins=[y1p.opt()],
                outs=[y1r.opt()],
            )
            y1in = l2p.tile([B_LOC, H], f32)
            nc.sync.dma_start(out=y1in[:], in_=y1r[:])

            # ---- transpose (16,256) -> packed (128, 32) o-major ----
            hpre = l2p.tile([128, 2 * B_LOC], f32)
            for t in range(2):
                pt = ps2.tile([128, B_LOC], f32, tag="tp")
                nc.tensor.transpose(pt[:], y1in[:, 128 * t : 128 * (t + 1)],
                                    ident[:B_LOC, :B_LOC])
                nc.vector.tensor_copy(hpre[:, B_LOC * t : B_LOC * (t + 1)],
                                      pt[:])

            # ---- selu: h = max(lam*y,0) + la*(exp(min(y,0)) - 1) ----
            W2C = 2 * B_LOC  # 32
            ymin = l2p.tile([128, W2C], f32)
            e1 = l2p.tile([128, W2C], f32)
            a1 = l2p.tile([128, W2C], f32)
            c1 = l2p.tile([128, W2C], f32)
            h2 = l2p.tile([128, W2C], f32)
            nc.vector.tensor_scalar(ymin[:], hpre[:], 0.0, None, OP.min)
            nc.scalar.activation(e1[:], ymin[:], AF.Exp)
            nc.vector.tensor_scalar(a1[:], hpre[:], LAM, 0.0, OP.mult, OP.max)
            nc.vector.tensor_scalar(c1[:], e1[:], LA, LA, OP.mult, OP.subtract)
            nc.vector.tensor_tensor(h2[:], a1[:], c1[:], OP.add)

            # ---- layer-2 features ----
            F2 = l2p.tile([128, K2 // 128 * B_LOC], f32)  # (128, 416)
            # silu(h) = h / (1 + exp(-h))
            e2 = l2p.tile([128, W2C], f32)
            d2 = l2p.tile([128, W2C], f32)
            nc.scalar.activation(e2[:], h2[:], AF.Exp, scale=-1.0)
            nc.vector.tensor_scalar(d2[:], e2[:], 1.0, None, OP.add)
            nc.vector.reciprocal(d2[:], d2[:])
            nc.vector.tensor_tensor(F2[:, 12 * W2C :], h2[:], d2[:], OP.mult)
            # u2 and batched relu^3 features over all 12 shifts
            u2 = l2p.tile([128, W2C], f32)
            nc.vector.tensor_scalar(u2[:], h2[:], 2.5, 5.5, OP.mult, OP.add)
            nc.vector.tensor_scalar(u2[:], u2[:], 12.0, None, OP.min)
            r2 = l2p.tile([128, 12 * W2C], f32)
            s2 = l2p.tile([128, 12 * W2C], f32)
            nc.vector.tensor_tensor(
                r2[:].rearrange("p (m c) -> p m c", m=12),
                u2[:].unsqueeze(1).broadcast_to((128, 12, W2C)),
                mbias[:].rearrange("p (m c) -> p m c", m=12),
                OP.subtract,
            )
            nc.vector.tensor_scalar(r2[:], r2[:], 0.0, None, OP.max)
            nc.vector.tensor_tensor(s2[:], r2[:], r2[:], OP.mult)
            nc.vector.tensor_tensor(F2[:, : 12 * W2C], s2[:], r2[:], OP.mult)

            # ---- layer-2 weights + matmul: 26 chunks -> (16, 10) ----
            w2sb = l2p.tile([128, NK2 * OUT], f32)  # (128, 260)
            nc.sync.dma_start(out=w2sb[:], in_=w2_d.ap())
            yps2 = ps2.tile([B_LOC, OUT], f32, tag="acc2")
            for j in range(NK2):
                nc.tensor.matmul(
                    yps2[:],
                    F2[:, B_LOC * j : B_LOC * (j + 1)],
                    w2sb[:, OUT * j : OUT * (j + 1)],
                    start=(j == 0),
                    stop=(j == NK2 - 1),
                )
            ysb = l2p.tile([B_LOC, OUT], f32)
            nc.vector.tensor_copy(ysb[:], yps2[:])
            nc.sync.dma_start(out=yp_d.ap(), in_=ysb[:])

    _legalize_waits(nc)
    return nc


_NC_CACHE = None


def _get_program():
    global _NC_CACHE
    if _NC_CACHE is None:
        _NC_CACHE = _build_program()
    return _NC_CACHE


def _prep_inputs(x, coef1, scale_base1, scale_sp1, coef2, scale_base2,
                 scale_sp2):
    W1q = _fold(coef1, scale_sp1)                      # (256, 3072, 12)
    W2q = _fold(coef2, scale_sp2)                      # (10, 256, 12)
    w2full = np.concatenate(
        [
            np.ascontiguousarray(W2q.transpose(2, 1, 0)).reshape(12 * H, OUT),
            np.ascontiguousarray(scale_base2.T).reshape(H, OUT),
        ],
        axis=0,
    )                                                   # (3328, 10)
    w2full = np.ascontiguousarray(
        w2full.reshape(NK2, 128, OUT).transpose(1, 0, 2)).reshape(128, NK2 * OUT)
    in_maps = []
    for c in range(NC_CORES):
        sl = slice(c * I_LOC, (c + 1) * I_LOC)
        w1c = np.concatenate(
            [
                np.ascontiguousarray(W1q[:, sl, :].transpose(2, 1, 0))
                .reshape(12 * I_LOC, H),
                np.ascontiguousarray(scale_base1[:, sl].T).reshape(I_LOC, H),
            ],
            axis=0,
        )                                               # (4992, 256)
        w1c = np.ascontiguousarray(
            w1c.reshape(NK1, 128, H).transpose(1, 0, 2)).reshape(128, NK1 * H)
        xtc = np.ascontiguousarray(
            x[:, sl].T.reshape(3, 128, B).transpose(1, 0, 2)).reshape(128, 3 * B)
        in_maps.append({"xt": xtc, "w1": w1c, "w2": w2full})
    return in_maps


def kernel(x, coef1, scale_base1, scale_sp1, coef2, scale_base2, scale_sp2,
           _trace=False, **_unused):
    x = np.asarray(x, np.float32)
    coef1 = np.asarray(coef1, np.float32)
    scale_base1 = np.asarray(scale_base1, np.float32)
    scale_sp1 = np.asarray(scale_sp1, np.float32)
    coef2 = np.asarray(coef2, np.float32)
    scale_base2 = np.asarray(scale_base2, np.float32)
    scale_sp2 = np.asarray(scale_sp2, np.float32)

    in_maps = _prep_inputs(x, coef1, scale_base1, scale_sp1, coef2,
                           scale_base2, scale_sp2)
    nc = _get_program()
    res = run_bass_kernel_spmd(nc, in_maps, list(range(NC_CORES)),
                               trace=_trace)
    out = np.concatenate([np.asarray(res.results[c]["yp"])
                          for c in range(NC_CORES)], axis=0)
    if _trace:
        return out, res
    return out


# revision 8
# speedup vs baseline: 1.4476x; 1.4205x over previous
"""Two-layer KAN (B-spline + silu base) fused Trainium2 kernel, 8-core SPMD.

Math: cubic B-spline basis on uniform grid [-2.2, 2.2] (h=0.4) rewritten as
relu(u-m)^3 features (u = 2.5*x + 5.5, clamped at 12), with the 5-tap stencil
[1,-4,6,-4,1]/6 folded into the spline weights host-side. Each KAN layer
becomes one dense matmul over 13 feature blocks (12 relu^3 + silu base).

Sharding: layer 1 contraction(in_dim)-parallel across 8 cores; partial
y1 (128,256) ReduceScatter(add) -> each core owns 16 batch rows; layer 2
batch-parallel with full contraction; host concatenates the 8 (16,10) shards.
"""

import ml_dtypes
import numpy as np
import concourse.bass as bass
import concourse.mybir as mybir
import concourse.tile as tile
from concourse.bass_utils import run_bass_kernel_spmd
from concourse.masks import make_identity
from concourse.vector_clock import ScopedClock

f32 = mybir.dt.float32
f32r = mybir.dt.float32r
bf16 = mybir.dt.bfloat16
AF = mybir.ActivationFunctionType
OP = mybir.AluOpType

NC_CORES = 8
B, IN, H, OUT, NB = 128, 3072, 256, 10, 8
I_LOC = IN // NC_CORES          # 384
NF = 13                         # 12 relu^3 features + silu base block
K1 = I_LOC * NF                 # 4992
NK1 = K1 // 128                 # 39
B_LOC = B // NC_CORES           # 16
K2 = H * NF                     # 3328
NK2 = K2 // 128                 # 26
LAM = 1.0507009873554805
ALPHA = 1.6732632423543772
LA = LAM * ALPHA
STENCIL = (np.array([1.0, -4.0, 6.0, -4.0, 1.0]) / 6.0).astype(np.float32)

# walrus codegen rejects instructions carrying more than one sem wait at the
# TileContext exit drain; split it into a chain of single-wait drains.
_WAIT_LIMIT = 1


def _patched_drain_and_barrier(self, tick_clock, wait_clock):
    nc = self.nc
    drain_inst = nc.sync.drain()
    wait_clock.add_sem_waits(
        drain_inst.ins, ScopedClock({None: tick_clock.global_clock})
    )
    si = drain_inst.ins.sync_info
    waits = list(si.on_wait) if si and si.on_wait else []
    if len(waits) > _WAIT_LIMIT:
        si.on_wait = waits[:_WAIT_LIMIT]
        for ofs in range(_WAIT_LIMIT, len(waits), _WAIT_LIMIT):
            extra = nc.sync.drain()
            chunk = waits[ofs : ofs + _WAIT_LIMIT]
            if extra.ins.sync_info is None:
                extra.ins.sync_info = mybir.SyncInfo(on_update=[], on_wait=chunk)
            else:
                extra.ins.sync_info.on_wait = chunk
    nc.all_engine_barrier()
    assert self.sems is not None
    popped = nc._tile_sem_poison_stack.pop()
    assert popped is self._sem_poison
    nc.clear_and_free_semaphores(list(self.sems.allocated().values()))
    nc.all_engine_barrier()


tile.TileContext._drain_and_barrier = _patched_drain_and_barrier


def _legalize_waits(nc, limit=1):
    """Split any instruction carrying >limit sem waits: move the overflow onto
    no-op instructions inserted immediately before it on the same engine."""
    n = 0
    for bbw in nc.bb_map.values():
        bb = bbw.bb
        i = 0
        while i < len(bb.instructions):
            inst = bb.instructions[i]
            si = inst.sync_info
            waits = list(si.on_wait) if si and si.on_wait else []
            if len(waits) > limit:
                si.on_wait = waits[-limit:]
                overflow = waits[:-limit]
                for ofs in range(0, len(overflow), limit):
                    nop = mybir.InstNoOp(name=f"legwait-{n}", engine=inst.engine,
                                         debug=inst.debug, ins=[], outs=[])
                    nop.sync_info = mybir.SyncInfo(
                        on_update=[], on_wait=overflow[ofs : ofs + limit])
                    nc.register_instruction(nop, overwrite=True)
                    bb.instructions.insert(i, nop)
                    n += 1
                    i += 1
            i += 1
    return n


def _fold(coef, ssp):
    """(O,I,8) spline coefs + per-edge scale -> (O,I,12) relu^3 weights."""
    O, I, _ = coef.shape
    cs = (coef * ssp[:, :, None]).astype(np.float32)
    W = np.zeros((O, I, 12), np.float32)
    for g in range(NB):
        for d in range(5):
            W[:, :, g + d] += cs[:, :, g] * STENCIL[d]
    return W


def _build_program():
    nc = bass.Bass("TRN2", target_bir_lowering=False, debug=False,
                   num_devices=NC_CORES)
    xt_d = nc.dram_tensor("xt", [128, 3 * B], f32, kind="ExternalInput")
    w1_d = nc.dram_tensor("w1", [128, NK1 * H], bf16, kind="ExternalInput")
    w2_d = nc.dram_tensor("w2", [128, NK2 * OUT], f32, kind="ExternalInput")
    yp_d = nc.dram_tensor("yp", [B_LOC, OUT], f32, kind="ExternalOutput")

    with tile.TileContext(nc) as tc:
        with (
            tc.tile_pool(name="constp", bufs=1) as constp,
            tc.tile_pool(name="xp", bufs=1) as xp,
            tc.tile_pool(name="fp", bufs=1) as fp,
            tc.tile_pool(name="wp", bufs=4) as wp,
            tc.tile_pool(name="sp", bufs=4) as sp,
            tc.tile_pool(name="l2p", bufs=1) as l2p,
            tc.tile_pool(name="ps1", bufs=1, space="PSUM") as ps1,
            tc.tile_pool(name="ps2", bufs=2, space="PSUM") as ps2,
            tc.tile_pool(name="dram", bufs=1, space="DRAM") as dram,
        ):
            # constants
            ident = constp.tile([128, 128], f32)
            make_identity(nc, ident)
            mbias = constp.tile([128, 12 * 2 * B_LOC], f32)  # (128, 384)
            for m in range(12):
                nc.vector.memset(mbias[:, 32 * m : 32 * (m + 1)], float(m))
            warm = constp.tile([1, 1], f32)

            # ---- layer 1: x^T load, u, features ----
            xt = xp.tile([128, 3 * 128], f32)
            nc.sync.dma_start(out=xt[:], in_=xt_d.ap())
            u = xp.tile([128, 3 * 128], f32)
            nc.vector.tensor_scalar(u[:], xt[:], 2.5, 5.5, OP.mult, OP.add)
            nc.vector.tensor_scalar(u[:], u[:], 12.0, None, OP.min)

            F = fp.tile([128, K1], bf16)
            nc.scalar.activation(F[:, 12 * I_LOC :], xt[:], AF.Silu)
            for m in range(12):
                r = sp.tile([128, I_LOC], f32, tag="r")
                s = sp.tile([128, I_LOC], f32, tag="s")
                nc.vector.tensor_scalar(r[:], u[:], float(m), 0.0,
                                        OP.subtract, OP.max)
                nc.scalar.activation(s[:], r[:], AF.Square)
                nc.vector.tensor_tensor(F[:, I_LOC * m : I_LOC * (m + 1)],
                                        s[:], r[:], OP.mult)
            # pre-warm Exp table while matmuls run
            nc.scalar.activation(warm[:], xt[:1, :1], AF.Exp)

            # ---- layer 1 matmul: 39 accumulating chunks ----
            y1ps = ps1.tile([128, H], f32)
            for i in range(13):
                wt = wp.tile([128, 3 * H], bf16, tag="w1")
                nc.sync.dma_start(
                    out=wt[:], in_=w1_d.ap()[:, 3 * H * i : 3 * H * (i + 1)])
                for s3 in range(3):
                    j = 3 * i + s3
                    nc.tensor.matmul(
                        y1ps[:],
                        F[:, 128 * j : 128 * (j + 1)],
                        wt[:, H * s3 : H * (s3 + 1)],
                        start=(j == 0),
                        stop=(j == NK1 - 1),
                    )
            y1sb = l2p.tile([128, H], f32)
            nc.vector.tensor_copy(y1sb[:], y1ps[:])

            # ---- ReduceScatter: each core gets its 16 batch rows ----
            y1p = dram.tile([B, H], f32)
            y1r = dram.tile([B_LOC, H], f32)
            nc.sync.dma_start(out=y1p[:], in_=y1sb[:])
            nc.gpsimd.collective_compute(
                "ReduceScatter",
                OP.add,
                replica_groups=[list(range(NC_CORES))],
                ins=[y1p.opt()],
                outs=[y1r.opt()],
            )
            y1in = l2p.tile([B_LOC, H], f32)
            nc.sync.dma_start(out=y1in[:], in_=y1r[:])

            # ---- transpose (16,256) -> packed (128, 32) o-major ----
            hpre = l2p.tile([128, 2 * B_LOC], f32)
            for t in range(2):
                pt = ps2.tile([128, B_LOC], f32, tag="tp")
                nc.tensor.transpose(pt[:], y1in[:, 128 * t : 128 * (t + 1)],
                                    ident[:B_LOC, :B_LOC])
                nc.vector.tensor_copy(hpre[:, B_LOC * t : B_LOC * (t + 1)],
                                      pt[:])

            # ---- selu: h = max(lam*y,0) + la*(exp(min(y,0)) - 1) ----
            W2C = 2 * B_LOC  # 32
            ymin = l2p.tile([128, W2C], f32)
            e1 = l2p.tile([128, W2C], f32)
            a1 = l2p.tile([128, W2C], f32)
            c1 = l2p.tile([128, W2C], f32)
            h2 = l2p.tile([128, W2C], f32)
            nc.vector.tensor_scalar(ymin[:], hpre[:], 0.0, None, OP.min)
            nc.scalar.activation(e1[:], ymin[:], AF.Exp)
            nc.vector.tensor_scalar(a1[:], hpre[:], LAM, 0.0, OP.mult, OP.max)
            nc.vector.tensor_scalar(c1[:], e1[:], LA, LA, OP.mult, OP.subtract)
            nc.vector.tensor_tensor(h2[:], a1[:], c1[:], OP.add)

            # ---- layer-2 features ----
            F2 = l2p.tile([128, K2 // 128 * B_LOC], f32)  # (128, 416)
            # silu(h) = h / (1 + exp(-h))
            e2 = l2p.tile([128, W2C], f32)
            d2 = l2p.tile([128, W2C], f32)
            nc.scalar.activation(e2[:], h2[:], AF.Exp, scale=-1.0)
            nc.vector.tensor_scalar(d2[:], e2[:], 1.0, None, OP.add)
            nc.vector.reciprocal(d2[:], d2[:])
            nc.vector.tensor_tensor(F2[:, 12 * W2C :], h2[:], d2[:], OP.mult)
            # u2 and batched relu^3 features over all 12 shifts
            u2 = l2p.tile([128, W2C], f32)
            nc.vector.tensor_scalar(u2[:], h2[:], 2.5, 5.5, OP.mult, OP.add)
            nc.vector.tensor_scalar(u2[:], u2[:], 12.0, None, OP.min)
            r2 = l2p.tile([128, 12 * W2C], f32)
            s2 = l2p.tile([128, 12 * W2C], f32)
            nc.vector.tensor_tensor(
                r2[:].rearrange("p (m c) -> p m c", m=12),
                u2[:].unsqueeze(1).broadcast_to((128, 12, W2C)),
                mbias[:].rearrange("p (m c) -> p m c", m=12),
                OP.subtract,
            )
            nc.vector.tensor_scalar(r2[:], r2[:], 0.0, None, OP.max)
            nc.vector.tensor_tensor(s2[:], r2[:], r2[:], OP.mult)
            nc.vector.tensor_tensor(F2[:, : 12 * W2C], s2[:], r2[:], OP.mult)

            # ---- layer-2 weights + matmul: 26 chunks -> (16, 10) ----
            w2sb = l2p.tile([128, NK2 * OUT], f32)  # (128, 260)
            nc.sync.dma_start(out=w2sb[:], in_=w2_d.ap())
            yps2 = ps2.tile([B_LOC, OUT], f32, tag="acc2")
            for j in range(NK2):
                nc.tensor.matmul(
                    yps2[:],
                    F2[:, B_LOC * j : B_LOC * (j + 1)],
                    w2sb[:, OUT * j : OUT * (j + 1)],
                    start=(j == 0),
                    stop=(j == NK2 - 1),
                )
            ysb = l2p.tile([B_LOC, OUT], f32)
            nc.vector.tensor_copy(ysb[:], yps2[:])
            nc.sync.dma_start(out=yp_d.ap(), in_=ysb[:])

    _legalize_waits(nc)
    return nc


_NC_CACHE = None


def _get_program():
    global _NC_CACHE
    if _NC_CACHE is None:
        _NC_CACHE = _build_program()
    return _NC_CACHE


def _prep_inputs(x, coef1, scale_base1, scale_sp1, coef2, scale_base2,
                 scale_sp2):
    W1q = _fold(coef1, scale_sp1)                      # (256, 3072, 12)
    W2q = _fold(coef2, scale_sp2)                      # (10, 256, 12)
    w2full = np.concatenate(
        [
            np.ascontiguousarray(W2q.transpose(2, 1, 0)).reshape(12 * H, OUT),
            np.ascontiguousarray(scale_base2.T).reshape(H, OUT),
        ],
        axis=0,
    )                                                   # (3328, 10)
    w2full = np.ascontiguousarray(
        w2full.reshape(NK2, 128, OUT).transpose(1, 0, 2)).reshape(128, NK2 * OUT)
    in_maps = []
    for c in range(NC_CORES):
        sl = slice(c * I_LOC, (c + 1) * I_LOC)
        w1c = np.concatenate(
            [
                np.ascontiguousarray(W1q[:, sl, :].transpose(2, 1, 0))
                .reshape(12 * I_LOC, H),
                np.ascontiguousarray(scale_base1[:, sl].T).reshape(I_LOC, H),
            ],
            axis=0,
        )                                               # (4992, 256)
        w1c = np.ascontiguousarray(
            w1c.reshape(NK1, 128, H).transpose(1, 0, 2)).reshape(128, NK1 * H)
        w1c = w1c.astype(ml_dtypes.bfloat16)
        xtc = np.ascontiguousarray(
            x[:, sl].T.reshape(3, 128, B).transpose(1, 0, 2)).reshape(128, 3 * B)
        in_maps.append({"xt": xtc, "w1": w1c, "w2": w2full})
    return in_maps


def kernel(x, coef1, scale_base1, scale_sp1, coef2, scale_base2, scale_sp2,
           _trace=False, **_unused):
    x = np.asarray(x, np.float32)
    coef1 = np.asarray(coef1, np.float32)
    scale_base1 = np.asarray(scale_base1, np.float32)
    scale_sp1 = np.asarray(scale_sp1, np.float32)
    coef2 = np.asarray(coef2, np.float32)
    scale_base2 = np.asarray(scale_base2, np.float32)
    scale_sp2 = np.asarray(scale_sp2, np.float32)

    in_maps = _prep_inputs(x, coef1, scale_base1, scale_sp1, coef2,
                           scale_base2, scale_sp2)
    nc = _get_program()
    res = run_bass_kernel_spmd(nc, in_maps, list(range(NC_CORES)),
                               trace=_trace)
    out = np.concatenate([np.asarray(res.results[c]["yp"])
                          for c in range(NC_CORES)], axis=0)
    if _trace:
        return out, res
    return out
